# revision 44
# baseline (speedup 1.0000x reference)
"""Trainium2 Bass kernel: causal self-attention with ALiBi.

Problem: B=4, S=2048, E=1024, H=16, D=64 (fp32 in/out).

Sharding (8 cores): core c -> (batch b = c//2, head-group g = c%2), 8 heads
per group. Each core computes QKV projection for its batch restricted to its
heads, attention for those (b, h), and a partial output projection over its
heads' dims. Host sums the two partials per batch and adds b_out.

Design (v2, from the 233us baseline):
 - Q and K are PAIR tiles [128, PAIRS, S]: a head pair's 2x64 dims live in
   the 128 partitions (Q pre-scaled by 1/sqrt(D)).
 - Scores per k-chunk are two ROW-TILED matmuls (K=64 contraction each):
   head0 in array rows 0-63, head1 in rows 64-127; concurrent in the PE.
 - PER-SLOT ALiBi window clipping (MARGIN=6): each slot s (head position)
   has its own window delta = max over the two core groups; head1 of a pair
   always has half the window of head0, so its score/exp/AV widths w1 <= w0
   and whole chunks can skip head1.  PSUM accumulate start=True clears the
   whole bank's has_written bits, and untouched columns get overwritten by
   their (always-present) diagonal chunk, so no full-width first chunk is
   needed.
 - Software-pipelined rounds as before (fill()), with PAIR HOOKS: the last
   round interleaves the tile-3 output projection per attention pair into
   the pending queue, accumulating pairs 0-2 into an SBUF partial so the
   tail is only pair 3's 8 matmuls + adds + DMAs.
 - QKV bias evacuation on the vector engine (tensor_scalar_add) instead of
   scalar, keeping the scalar queue free for the exp chain.
 - Output written bf16 (host sums partials in fp32), out-DMAs on the gpsimd
   queue.
 - Normalization: AV psum carries an l row (ones column in V); 1/l broadcast
   across partitions via a DRAM round-trip DMA, deferred into later chunk
   slots; the final pair uses a PE outer-product broadcast instead.
"""

import sys

if '/opt/trn_rl_repo' not in sys.path:
    sys.path.insert(0, '/opt/trn_rl_repo')

import numpy as np
import ml_dtypes

import concourse.bass as bass
import concourse.tile as tile
from concourse import bacc, mybir
from concourse import bass_utils

BF16 = mybir.dt.bfloat16
F32 = mybir.dt.float32
AF = mybir.ActivationFunctionType

B, S, E, H, D = 4, 2048, 1024, 16, 64
NH = 8          # heads per core
N_CORES = 8
PAIRS = NH // 2
P = 128
ST = 512        # s/q tile (free dim)
NST = S // ST   # 4
NSC = S // P    # 16 s-chunks
NCC = E // P    # 8 contraction chunks
EMW2 = 256      # EM table width (pairs 2-3; max col = 128 + slot4 delta)

# ALiBi window: keys further than MARGIN/slope below the diagonal contribute
# exp(alibi) < e^-MARGIN of the max element and are skipped (chunk skip for
# whole chunks, column clip within a chunk, per slot).  Head h's slope is
# 2^-((h+1)/2).  Heads are assigned to the two core groups by descending
# window so both groups' slot s have similar cost; clip widths use the
# per-slot max over the two groups (the program is shared by all cores).
MARGIN = 5.0

SLOT_HEADS_A = [15, 13, 11, 9, 7, 5, 3, 1]
SLOT_HEADS_B = [14, 12, 10, 8, 6, 4, 2, 0]

import math as _m


def _delta_h(h):
    return MARGIN / (2.0 ** (-(h + 1) / 2.0))


def _slot_delta(s):
    return int(_m.ceil(max(_delta_h(SLOT_HEADS_A[s]), _delta_h(SLOT_HEADS_B[s]))))


def _jlo(h, i):
    delta = _delta_h(h)
    return max(0, _m.ceil((ST * i - delta - (P - 1)) / P))


def k_chunks_for_pair(pr, i):
    lo = min(_jlo(h, i) for s in (2 * pr, 2 * pr + 1)
             for h in (SLOT_HEADS_A[s], SLOT_HEADS_B[s]))
    return list(range(lo, 4 * i + 4))


def _slopes():
    x = (2.0 ** 8) ** (1.0 / H)
    return np.array([1.0 / x ** (i + 1) for i in range(H)], dtype=np.float64)


def _build_em(heads):
    """EM[p, si, c] = exp(slope_h*m)*[m<=0], m = k-c, h = heads[4+si]."""
    slopes = _slopes()
    k = np.arange(P, dtype=np.float64)[:, None]
    c = np.arange(EMW2, dtype=np.float64)[None, :]
    m = k - c
    em = np.zeros((P, 4, EMW2), dtype=np.float32)
    for si in range(4):
        h = heads[4 + si]
        em[:, si, :] = (np.exp(slopes[h] * m) * (m <= 0)).astype(np.float32)
    return em.astype(ml_dtypes.bfloat16)


def _build_bias_tabs(heads):
    """ALiBi bias table for the bias-route pairs 0-1 (slots 0-3), scaled
    relative to r0 = q-tile end (the per-q factor cancels in softmax):
      btab[k, s, t] = slope * (k + 128*(t-12) - 511)    (t_idx = j-4i+12)
    (the diagonal-chunk emg table is built on-chip from slopes4)
    """
    slopes = _slopes()
    kk = np.arange(P, dtype=np.float64)
    btab = np.zeros((P, 4, 12), dtype=np.float32)
    for s in range(4):
        sl = slopes[heads[s]]
        for t in range(12):
            btab[:, s, t] = (sl * (kk + 128 * (t - 12) - 511)).astype(
                np.float32)
    return btab


_PROG_CACHE = {}


def _build_program():
    if 'prog' in _PROG_CACHE:
        return _PROG_CACHE['prog']

    nc = bacc.Bacc("TRN2", target_bir_lowering=False, debug=False,
                   num_devices=N_CORES)

    xT = nc.dram_tensor("xT", [NST, P, NCC, ST], BF16, kind="ExternalInput")
    wqk = nc.dram_tensor("wqk", [P, 2 * PAIRS, NCC, P], BF16, kind="ExternalInput")
    wv = nc.dram_tensor("wv", [P, NCC, NH * D], BF16, kind="ExternalInput")
    wout = nc.dram_tensor("wout", [P, PAIRS, E], BF16, kind="ExternalInput")
    bqk = nc.dram_tensor("bqk", [P, NCC], F32, kind="ExternalInput")
    bv = nc.dram_tensor("bv", [NH * D], BF16, kind="ExternalInput")
    em = nc.dram_tensor("em", [P, 4, EMW2], BF16, kind="ExternalInput")
    btab = nc.dram_tensor("btab", [P, 4, 12], F32, kind="ExternalInput")
    slopes4 = nc.dram_tensor("slopes4", [4], F32, kind="ExternalInput")
    out = nc.dram_tensor("out", [S, E], BF16, kind="ExternalOutput")

    with tile.TileContext(nc) as tc:
        with (
            tc.tile_pool(name="persist", bufs=1) as pp,
            tc.tile_pool(name="xt", bufs=3) as xtp,
            tc.tile_pool(name="pm", bufs=4) as pmp,
            tc.tile_pool(name="misc", bufs=2) as misc,
            tc.tile_pool(name="osb", bufs=2) as osb,
            tc.tile_pool(name="drp", bufs=8, space="DRAM") as drp,
        ):
            p1 = tc.tile_pool(name="qkvps", bufs=2, space="PSUM")
            qkps = p1.__enter__()
            p2a = tc.tile_pool(name="scps", bufs=2, space="PSUM")
            scps = p2a.__enter__()
            p2b = tc.tile_pool(name="avps", bufs=1, space="PSUM")
            avps = p2b.__enter__()

            # --- PE warmup first: the HAM clock gate needs ~3.4us of
            # sustained matmul activity to lift the PE from 1.2 to 2.4 GHz;
            # burn the DMA-bound startup on dummy matmuls (16 independent
            # psum slots so WAW deps don't serialize them).  Later DMA-wait
            # gaps are shorter than the ~3.4us MID window, so the clock
            # stays high once flipped.
            warm = pp.tile([P, P], BF16, tag="warm")
            nc.vector.memset(warm, 0.0)
            wps0 = qkps.tile([P, ST], F32, tag="ps", name="wps")
            wps1 = qkps.tile([P, ST], F32, tag="ps", name="wps")
            for _w in range(128):
                t_ = wps0 if _w % 2 == 0 else wps1
                c_ = ((_w // 2) % 4) * P
                nc.tensor.matmul(t_[:, c_:c_ + P], lhsT=warm,
                                 rhs=warm, start=True, stop=True)

            # --- weight / table DMAs, ordered so phase-1 can start ASAP ---
            # wqk is m-chunk-major on the host so the first QKV matmul only
            # waits for a 0.25MB DMA; em must land before attention round 0.
            wqk_sb = pp.tile([P, 2 * PAIRS, NCC, P], BF16, tag="wqk")
            bqk_sb = pp.tile([P, NCC], F32, tag="bqk")
            nc.sync.dma_start(wqk_sb[:, 0:1], wqk.ap()[:, 0:1])
            nc.gpsimd.dma_start(wqk_sb[:, 1:2], wqk.ap()[:, 1:2])
            slope_bc = pp.tile([P, 4], F32, tag="slbc")
            nc.gpsimd.dma_start(
                slope_bc, slopes4.ap()[None, :].to_broadcast([P, 4]))
            nc.sync.dma_start(bqk_sb, bqk.ap())
            nc.sync.dma_start(wqk_sb[:, 2:3], wqk.ap()[:, 2:3])
            nc.gpsimd.dma_start(wqk_sb[:, 3:4], wqk.ap()[:, 3:4])
            nc.sync.dma_start(wqk_sb[:, 4:5], wqk.ap()[:, 4:5])
            nc.gpsimd.dma_start(wqk_sb[:, 5:6], wqk.ap()[:, 5:6])
            nc.sync.dma_start(wqk_sb[:, 6:7], wqk.ap()[:, 6:7])
            nc.gpsimd.dma_start(wqk_sb[:, 7:8], wqk.ap()[:, 7:8])
            wv_sb = pp.tile([P, NCC, NH * D], BF16, tag="wv")
            nc.gpsimd.dma_start(wv_sb[:, 0:NCC // 2], wv.ap()[:, 0:NCC // 2])
            nc.sync.dma_start(wv_sb[:, NCC // 2:], wv.ap()[:, NCC // 2:])
            em_sb = pp.tile([P, 4, EMW2], BF16, tag="em")
            nc.sync.dma_start(em_sb, em.ap())
            btab_sb = pp.tile([P, 4, 12], F32, tag="btab")
            nc.sync.dma_start(btab_sb, btab.ap())
            ones_bf = pp.tile([1, D], BF16, tag="ones")
            nc.vector.memset(ones_bf, 1.0)
            wout_sb = pp.tile([P, PAIRS, E], BF16, tag="wout")

            bv_bc = pp.tile([P, NH * D], BF16, tag="bvbc")
            q_sb = pp.tile([P, PAIRS, S], BF16, tag="q")
            k_sb = pp.tile([P, PAIRS, S], BF16, tag="k")
            v_sb = pp.tile([P, NSC, NH, D + 1], BF16, tag="v")
            attn_sb = pp.tile([P, PAIRS, S], BF16, tag="attn")

            nc.vector.memset(v_sb[:, :, :, D:D + 1], 1.0)


            # --- on-chip emg build: emg[p,t,s,c] = exp(sl_s*(p-511+128t))
            # * tril[p<=c].  Saves 2MB of startup HBM traffic; runs on the
            # otherwise-idle gpsimd/vector engines while weights stream in.
            ramp = pp.tile([P, ST], F32, tag="ramp")
            nc.gpsimd.iota(ramp, pattern=[[-1, ST]], base=0,
                           channel_multiplier=1,
                           allow_small_or_imprecise_dtypes=True)
            tril = pp.tile([P, ST], BF16, tag="tril")
            nc.vector.tensor_scalar(out=tril, in0=ramp, scalar1=0.0,
                                    scalar2=None,
                                    op0=mybir.AluOpType.is_le)
            # identity for the tail's PE partition-move (odd attn half)
            ident = pp.tile([D, D], BF16, tag="ident")
            nc.vector.tensor_scalar(out=ident, in0=ramp[0:D, 0:D],
                                    scalar1=0.0, scalar2=None,
                                    op0=mybir.AluOpType.is_equal)
            tio = pp.tile([P, 4], F32, tag="tio")
            nc.gpsimd.iota(tio, pattern=[[128, 4]], base=-511,
                           channel_multiplier=1,
                           allow_small_or_imprecise_dtypes=True)
            evec = pp.tile([P, 4, 4], F32, tag="evec")  # [p, s, t]
            emg_sb = pp.tile([P, 4, 4, ST], BF16, tag="emg")  # [p, t, s, c]
            for s_ in range(4):
                nc.scalar.activation(evec[:, s_, :], tio, AF.Exp,
                                     scale=slope_bc[:, s_:s_ + 1])
            for t_ in range(4):
                for s_ in range(4):
                    nc.vector.tensor_scalar_mul(
                        emg_sb[:, t_, s_, :], tril, evec[:, s_, t_:t_ + 1])


            xt_tiles = {}

            def issue_xt_dma(i):
                xt = xtp.tile([P, NCC, ST], BF16, tag="xt")
                nch = 4 if i == 0 else 2
                for cc in range(nch):
                    sl_ = slice(cc * NCC // nch, (cc + 1) * NCC // nch)
                    nc.scalar.dma_start(xt[:, sl_], xT.ap()[i][:, sl_])
                xt_tiles[i] = xt
                if i == 0:
                    # tiny, needed by the first V bias add (~30us in); the
                    # scalar queue is otherwise idle here.
                    nc.scalar.dma_start(
                        bv_bc, bv.ap()[None, :].to_broadcast([P, NH * D]))


            def dummy_mms(k):
                """HAM warm-keeping: tiny matmuls on scratch data so DMA
                waits in the real stream don't let the PE clock drop."""
                for _ in range(k):
                    wps = qkps.tile([P, ST], F32, tag="ps", name="wps")
                    nc.tensor.matmul(wps[0:D, 0:D], lhsT=warm[:, 0:D],
                                     rhs=warm[:, 0:D], start=True, stop=True)

            def qkv_steps(i, dummies=0):
                """Per-step closures for tile i's QKV projections."""
                xt = xt_tiles.pop(i)
                sl = slice(i * ST, (i + 1) * ST)
                st = {}
                steps = []
                for m in range(2 * PAIRS):
                    for c in range(NCC):
                        def f(m=m, c=c):
                            if c == 0:
                                st[m] = qkps.tile([P, ST], F32, tag="ps", name="ps")
                            nc.tensor.matmul(
                                st[m], lhsT=wqk_sb[:, m, c, :],
                                rhs=xt[:, c, :],
                                start=(c == 0), stop=(c == NCC - 1))
                        steps.append(f)

                    def fa(m=m):
                        pr = m // 2
                        dst = q_sb if m % 2 == 0 else k_sb
                        if m % 2 == 0:
                            nc.vector.tensor_scalar_add(
                                dst[:, pr, sl], st.pop(m), bqk_sb[:, m:m + 1])
                        else:
                            nc.scalar.activation(
                                dst[:, pr, sl], st.pop(m), AF.Identity,
                                bias=bqk_sb[:, m:m + 1])
                        if dummies:
                            dummy_mms(dummies)
                    steps.append(fa)
                for s4 in range(4):
                    sc = 4 * i + s4
                    for c in range(NCC):
                        def f(s4=s4, c=c):
                            if c == 0:
                                st['v'] = qkps.tile([P, NH * D], F32, tag="ps", name="ps")
                            nc.tensor.matmul(
                                st['v'], lhsT=xt[:, c, s4 * P:(s4 + 1) * P],
                                rhs=wv_sb[:, c, :],
                                start=(c == 0), stop=(c == NCC - 1))
                        steps.append(f)

                    def fv(sc=sc):
                        vp = st.pop('v')
                        nc.vector.tensor_add(
                            out=v_sb[:, sc, :, 0:D],
                            in0=vp.rearrange("p (h d) -> p h d", h=NH),
                            in1=bv_bc.rearrange("p (h d) -> p h d", h=NH))
                        if dummies:
                            dummy_mms(dummies)
                    steps.append(fv)
                return steps

            def outproj_steps(i):
                """Per-step closures for tile i's output projection."""
                st = {}
                steps = []
                for qc in range(4 * i, 4 * i + 4):
                    for n in range(E // ST):
                        for p_ in range(PAIRS):
                            def f(qc=qc, n=n, p_=p_):
                                if p_ == 0:
                                    st['op'] = qkps.tile([P, ST], F32, tag="ps", name="ps")
                                nc.tensor.matmul(
                                    st['op'],
                                    lhsT=attn_sb[:, p_, qc * P:(qc + 1) * P],
                                    rhs=wout_sb[:, p_, n * ST:(n + 1) * ST],
                                    start=(p_ == 0), stop=(p_ == PAIRS - 1))
                            steps.append(f)

                        def fo(qc=qc, n=n):
                            op = st.pop('op')
                            ot = osb.tile([P, ST], BF16, tag="ot")
                            # alternate the psum-evacuation engine so neither
                            # the scalar nor the vector queue serializes the
                            # outproj group pipeline; the tile-3 tail leans
                            # on scalar (idle once the exps are done) to keep
                            # vector free for the last normalization.
                            if (qc + n) % 2 == 0:
                                nc.vector.tensor_copy(ot, op)
                            else:
                                nc.scalar.activation(ot, op, AF.Copy)
                            nc.gpsimd.dma_start(
                                out.ap()[qc * P:(qc + 1) * P,
                                         n * ST:(n + 1) * ST], ot)
                        steps.append(fo)
                return steps

            op3_state = {}

            def op3_run():
                """Tile-3 output projection with all 8 psum banks: the 24
                pair-0..2 matmuls go in flight first (their banks free up in
                readiness order: ps after the 1/l recips, scp after the last
                exps, av after the final normalization muls), covering the
                latency of pair 3's normalization; then the 8 pair-3 matmuls
                + evacuations drain the groups."""
                groups = [(qc, n) for qc in range(12, 16) for n in range(2)]
                tiles = {}
                bigs = {}
                dmaq = [nc.gpsimd, nc.sync, nc.scalar]

                def finish(g, qc, n):
                    t = tiles[g]
                    nc.tensor.matmul(
                        t, lhsT=attn_sb[:, 3, qc * P:(qc + 1) * P],
                        rhs=wout_sb[:, 3, n * ST:(n + 1) * ST],
                        start=False, stop=True)
                    ot = osb.tile([P, ST], BF16, tag="ot")
                    if g % 2 == 0:
                        nc.vector.tensor_copy(ot, t)
                    else:
                        nc.scalar.activation(ot, t, AF.Copy)
                    dmaq[g % 3].dma_start(
                        out.ap()[qc * P:(qc + 1) * P,
                                 n * ST:(n + 1) * ST], ot)

                dummy_mms(8)
                for g, (qc, n) in enumerate(groups[:7]):
                    if g < 2:
                        t = qkps.tile([P, ST], F32, tag="ps", name="ps")
                    elif g < 6:
                        if g % 2 == 0:
                            big = scps.tile([P, 2 * ST], F32, tag="scp",
                                            name="scp")
                            bigs[g] = big
                            t = big[:, 0:ST]
                        else:
                            t = bigs[g - 1][:, ST:2 * ST]
                    else:
                        t = avps.tile([P, ST], F32, tag="av0", name="avt")
                    tiles[g] = t
                    for p_ in range(3):
                        nc.tensor.matmul(
                            t, lhsT=attn_sb[:, p_, qc * P:(qc + 1) * P],
                            rhs=wout_sb[:, p_, n * ST:(n + 1) * ST],
                            start=(p_ == 0), stop=False)
                # odd half of pair 3 to partitions 64-127 (PE identity
                # matmul into the av1 bank, whose only outstanding reader
                # is the just-emitted fast-path mul -- no pool cycle)
                atmp = op3_state.pop('atmp')
                qsl = op3_state.pop('qsl')
                mv = avps.tile([P, ST], F32, tag="av1", name="avt")
                nc.tensor.matmul(mv[D:P, :], lhsT=ident, rhs=atmp,
                                 start=True, stop=True)
                nc.vector.tensor_copy(attn_sb[D:P, 3, qsl], mv[D:P, :])
                for g, (qc, n) in enumerate(groups[:7]):
                    finish(g, qc, n)
                # the 8th group reuses the av1 bank right after the
                # partition-move copy releases it
                g, (qc, n) = 7, groups[7]
                t = avps.tile([P, ST], F32, tag="av1", name="avt")
                tiles[g] = t
                for p_ in range(4):
                    nc.tensor.matmul(
                        t, lhsT=attn_sb[:, p_, qc * P:(qc + 1) * P],
                        rhs=wout_sb[:, p_, n * ST:(n + 1) * ST],
                        start=(p_ == 0), stop=(p_ == 3))
                ot = osb.tile([P, ST], BF16, tag="ot")
                nc.scalar.activation(ot, t, AF.Copy)
                dmaq[1].dma_start(
                    out.ap()[qc * P:(qc + 1) * P,
                             n * ST:(n + 1) * ST], ot)

            def attn_round(i, filler, last=False, reserve=0):
                """Attention for tile i, draining `filler` steps between
                matmuls to keep the PE busy through the exp/EM dep chain."""
                qsl = slice(i * ST, (i + 1) * ST)
                nslots = 2 * sum(len(k_chunks_for_pair(pr, i))
                                 for pr in range(PAIRS)) + 2 * PAIRS
                pending = []
                tail_steps = filler[len(filler) - reserve:] if reserve else []
                if reserve:
                    del filler[len(filler) - reserve:]
                slots_left = [nslots]

                def fill():
                    if pending:
                        pending.pop(0)()
                    k = -(-len(filler) // slots_left[0]) if filler else 0
                    for _ in range(k):
                        filler.pop(0)()
                    slots_left[0] -= 1

                for pr in range(PAIRS):
                    s0 = 2 * pr
                    d0 = _slot_delta(s0)
                    d1 = _slot_delta(s0 + 1)
                    av0 = avps.tile([P, ST], F32, tag="av0")
                    av1 = avps.tile([P, ST], F32, tag="av1")
                    js = k_chunks_for_pair(pr, i)
                    av_prev = [None]
                    first1 = [True]
                    for jj, j in enumerate(js):
                        # causal clip below the diagonal; per-slot ALiBi
                        # window clip above it.  Untouched psum columns are
                        # covered by the diagonal chunks (accumulate start
                        # clears the whole bank; unwritten elements are
                        # overwritten on first touch).
                        qlo = max(0, P * j - ST * i)
                        qhi0 = min(ST, P * j + P + d0 - ST * i)
                        qhi1 = min(ST, P * j + P + d1 - ST * i)
                        w0 = qhi0 - qlo
                        w1 = max(0, qhi1 - qlo)
                        qs0 = slice(i * ST + qlo, i * ST + qhi0)
                        qs1 = slice(i * ST + qlo, i * ST + max(qlo, qhi1))
                        jsl = slice(j * P, (j + 1) * P)
                        scp = scps.tile([P, 2 * ST], F32, tag="scp")
                        scp3 = scp.rearrange("p (t st) -> p t st", t=2)
                        # row-tiled pair: head0 in array rows 0-63, head1 in
                        # rows 64-127 -- concurrent in the PE array.
                        nc.tensor.matmul(
                            scp[:, 0:w0], lhsT=k_sb[0:D, pr, jsl],
                            rhs=q_sb[0:D, pr, qs0], start=True, stop=True)
                        if w1 > 0:
                            nc.tensor.matmul(
                                scp[:, ST:ST + w1], lhsT=k_sb[D:P, pr, jsl],
                                rhs=q_sb[D:P, pr, qs1], start=True, stop=True)
                        fill()
                        pm = pmp.tile([P, 2 * ST], BF16, tag="pm")
                        pm3 = pm.rearrange("p (t st) -> p t st", t=2)
                        if pr < 2 and j < 4 * i:
                            # below-diagonal chunk, small-slope pair: fold
                            # the ALiBi bias (relative to r0 = tile end)
                            # into the exp's per-partition bias; the per-q
                            # factor exp(slope*(q-r0)) cancels in num/l.
                            tb = j - 4 * i + 12
                            nc.scalar.activation(
                                pm3[:, 0, 0:w0], scp3[:, 0, 0:w0], AF.Exp,
                                bias=btab_sb[:, s0, tb:tb + 1])
                            if w1 > 0:
                                nc.scalar.activation(
                                    pm3[:, 1, 0:w1], scp3[:, 1, 0:w1], AF.Exp,
                                    bias=btab_sb[:, s0 + 1, tb:tb + 1])
                            av_rhs = pm
                        elif pr < 2:
                            # diagonal chunk of a small-slope pair: exp then
                            # multiply with the g-scaled masked table
                            # exp(slope*(k-511+128t))*tril.
                            pm2 = pmp.tile([P, 2 * ST], BF16, tag="pm2")
                            pm23 = pm2.rearrange("p (t st) -> p t st", t=2)
                            td = j - 4 * i
                            if w1 >= w0 - 352 and w1 > 0:
                                nc.scalar.activation(pm3[:, :, 0:w0],
                                                     scp3[:, :, 0:w0], AF.Exp)
                                nc.vector.tensor_mul(
                                    pm23[:, :, 0:w0], pm3[:, :, 0:w0],
                                    emg_sb[:, td, s0:s0 + 2, 0:w0])
                            else:
                                nc.scalar.activation(pm3[:, 0, 0:w0],
                                                     scp3[:, 0, 0:w0], AF.Exp)
                                nc.vector.tensor_mul(
                                    pm23[:, 0, 0:w0], pm3[:, 0, 0:w0],
                                    emg_sb[:, td, s0, 0:w0])
                                if w1 > 0:
                                    nc.scalar.activation(
                                        pm3[:, 1, 0:w1], scp3[:, 1, 0:w1],
                                        AF.Exp)
                                    nc.vector.tensor_mul(
                                        pm23[:, 1, 0:w1], pm3[:, 1, 0:w1],
                                        emg_sb[:, td, s0 + 1, 0:w1])
                            av_rhs = pm2
                        else:
                            pm2 = pmp.tile([P, 2 * ST], BF16, tag="pm2")
                            pm23 = pm2.rearrange("p (t st) -> p t st", t=2)
                            a0 = qlo - P * j + ST * i
                            if w1 >= w0 - 352 and w1 > 0:
                                nc.scalar.activation(pm3[:, :, 0:w0],
                                                     scp3[:, :, 0:w0], AF.Exp)
                                nc.vector.tensor_mul(
                                    pm23[:, :, 0:w0], pm3[:, :, 0:w0],
                                    em_sb[:, s0 - 4:s0 - 2, a0:a0 + w0])
                            else:
                                nc.scalar.activation(pm3[:, 0, 0:w0],
                                                     scp3[:, 0, 0:w0], AF.Exp)
                                nc.vector.tensor_mul(
                                    pm23[:, 0, 0:w0], pm3[:, 0, 0:w0],
                                    em_sb[:, s0 - 4, a0:a0 + w0])
                                if w1 > 0:
                                    nc.scalar.activation(
                                        pm3[:, 1, 0:w1], scp3[:, 1, 0:w1],
                                        AF.Exp)
                                    nc.vector.tensor_mul(
                                        pm23[:, 1, 0:w1], pm3[:, 1, 0:w1],
                                        em_sb[:, s0 - 3, a0:a0 + w1])
                            av_rhs = pm2
                        # one-chunk software pipeline: emit chunk c's AV
                        # after chunk c+1's scores so the in-order PE queue
                        # doesn't block on c's exp/mul chain.
                        st1 = first1[0] and w1 > 0
                        if w1 > 0:
                            first1[0] = False

                        def av_mm(jj=jj, j=j, qlo=qlo, qhi0=qhi0, qhi1=qhi1,
                                  w0=w0, w1=w1, av_rhs=av_rhs, st1=st1,
                                  lastc=(jj == len(js) - 1)):
                            nc.tensor.matmul(
                                av0[0:D + 1, qlo:qhi0],
                                lhsT=v_sb[:, j, s0, :],
                                rhs=av_rhs[:, 0:w0],
                                start=(jj == 0), stop=lastc)
                            if w1 > 0:
                                nc.tensor.matmul(
                                    av1[0:D + 1, qlo:qhi1],
                                    lhsT=v_sb[:, j, s0 + 1, :],
                                    rhs=av_rhs[:, ST:ST + w1],
                                    start=st1, stop=lastc)
                        if av_prev[0] is not None:
                            av_prev[0]()
                        av_prev[0] = av_mm
                        fill()
                    av_prev[0]()
                    # evacuate AV psum (incl. the l row) promptly on the
                    # scalar engine so the next pair can reuse the banks;
                    # the rest of the normalization (1/l broadcast via DRAM
                    # round trip, final mul) is deferred into the next
                    # pair's chunk slots to keep the vector queue flowing.
                    fast = last and pr == PAIRS - 1
                    if fast:
                        # last pair before the tail outproj: broadcast 1/l
                        # via PE outer products instead of the DRAM round
                        # trip, normalize straight out of the AV psum (the
                        # banks are not reused after this pair), and move
                        # the odd half to partitions 64-127 with a PE
                        # identity matmul instead of a DMA round trip --
                        # the tile-3 outproj tail is gated on these writes.
                        lbps = []
                        for odd, av in ((0, av0), (1, av1)):
                            lcast = misc.tile([1, ST], BF16, tag=f"lc{odd}",
                                              name="lc")
                            nc.scalar.activation(lcast, av[D:D + 1, :],
                                                 AF.Copy)
                            lbp = qkps.tile([P, ST], F32, tag="ps", name="ps")
                            nc.tensor.matmul(lbp[0:D, :],
                                             lhsT=ones_bf[0:1, :],
                                             rhs=lcast, start=True, stop=True)
                            lbps.append(lbp)
                        for odd, (av, lbp) in enumerate(
                                ((av0, lbps[0]), (av1, lbps[1]))):
                            lb = misc.tile([D, ST], F32, tag=f"lbc{odd}",
                                           name="lbc")
                            nc.vector.reciprocal_approx_fast(out=lb,
                                                             in_=lbp[0:D, :])
                            if not odd:
                                nc.vector.tensor_mul(attn_sb[0:D, pr, qsl],
                                                     av[0:D, :], lb)
                            else:
                                atmp = misc.tile([D, ST], BF16, tag="atmp",
                                                 name="atmp")
                                nc.vector.tensor_mul(atmp, av[0:D, :], lb)
                                # the partition-move of this half to
                                # attn_sb[64:128] is emitted inside op3_run
                                # (after its 24-matmul sweep) so it does not
                                # stall the in-order PE queue here.
                                op3_state['atmp'] = atmp
                                op3_state['qsl'] = qsl
                            fill()
                        continue
                    for odd, av in ((0, av0), (1, av1)):
                        avs = misc.tile([D + 1, ST], F32, tag=f"avs{odd}",
                                        name="avs")
                        if odd and not last:
                            nc.vector.tensor_copy(avs, av[0:D + 1, :])
                        else:
                            nc.scalar.activation(avs, av[0:D + 1, :], AF.Copy)
                        stn = {}

                        def s1(odd=odd, avs=avs, stn=stn):
                            scratch = drp.tile([1, ST], F32, tag=f"lscr{odd}",
                                               name="lscr")
                            nc.sync.dma_start(scratch, avs[D:D + 1, :])
                            lb = misc.tile([D, ST], F32, tag=f"lbc{odd}",
                                           name="lbc")
                            nc.sync.dma_start(
                                lb,
                                bass.AP(tensor=scratch.tensor,
                                        offset=scratch.offset,
                                        ap=[[0, D]] + scratch.ap[1:]))
                            stn['lb'] = lb

                        def s2(stn=stn):
                            nc.vector.reciprocal_approx_fast(out=stn['lb'],
                                                             in_=stn['lb'])

                        def s3(odd=odd, avs=avs, stn=stn, pr=pr):
                            lb = stn.pop('lb')
                            if not odd:
                                nc.vector.tensor_mul(attn_sb[0:D, pr, qsl],
                                                     avs[0:D, :], lb)
                            else:
                                atmp = misc.tile([D, ST], BF16, tag="atmp",
                                                 name="atmp")
                                nc.vector.tensor_mul(atmp, avs[0:D, :], lb)
                                nc.sync.dma_start(attn_sb[D:P, pr, qsl], atmp)

                        pending.extend([s1, s2, s3])
                        fill()
                # interleave the remaining normalization chain with reserved
                # filler so the PE is not idle across the round boundary; in
                # the last round add warm-keeping dummies so the clock stays
                # high through the normalization drain into the outproj tail.
                filler.extend(tail_steps)
                while pending:
                    pending.pop(0)()
                    if last:
                        dummy_mms(4)
                    if filler:
                        filler.pop(0)()
                while filler:
                    filler.pop(0)()

            # software-pipelined schedule:
            #   r0: qkv(0);  r1: attn(0)+qkv(1);  r2: attn(1)+qkv(2)
            #   r3: attn(2)+qkv(3)+outproj(0);  r4: attn(3)+outproj(1..3)
            issue_xt_dma(0)
            issue_xt_dma(1)
            for f in qkv_steps(0):
                f()
            issue_xt_dma(2)
            nc.gpsimd.dma_start(wout_sb, wout.ap())
            attn_round(0, qkv_steps(1), reserve=10)
            issue_xt_dma(3)
            attn_round(1, qkv_steps(2), reserve=10)
            attn_round(2, qkv_steps(3) + outproj_steps(0), reserve=10)
            attn_round(3, outproj_steps(1) + outproj_steps(2), last=True,
                       reserve=16)
            op3_run()
            p2b.__exit__(None, None, None)
            p2a.__exit__(None, None, None)
            p1.__exit__(None, None, None)

    nc.compile()
    _PROG_CACHE['prog'] = nc
    return nc


def _head_groups():
    return [SLOT_HEADS_A, SLOT_HEADS_B]


def _prep_core_inputs(x, W_qkv, b_qkv, W_out, b_out):
    """Build the 8 per-core input dicts (host-side shard + transform)."""
    groups = _head_groups()
    bf = ml_dtypes.bfloat16
    per_group = []
    for heads in groups:
        qcols = []
        kcols = []
        bqk_l = []
        for p in range(PAIRS):
            h0, h1 = heads[2 * p], heads[2 * p + 1]
            wq = np.concatenate([W_qkv[:, h0 * D:(h0 + 1) * D],
                                 W_qkv[:, h1 * D:(h1 + 1) * D]], axis=1) / 8.0
            wk = np.concatenate([W_qkv[:, E + h0 * D:E + (h0 + 1) * D],
                                 W_qkv[:, E + h1 * D:E + (h1 + 1) * D]], axis=1)
            qcols.append(wq)
            kcols.append(wk)
            bqk_l.append(np.concatenate([b_qkv[h0 * D:(h0 + 1) * D],
                                         b_qkv[h1 * D:(h1 + 1) * D]]) / 8.0)
            bqk_l.append(np.concatenate([b_qkv[E + h0 * D:E + (h0 + 1) * D],
                                         b_qkv[E + h1 * D:E + (h1 + 1) * D]]))
        # interleave Q-pair / K-pair chunks: m even = Q, m odd = K
        wqk_l = np.empty((E, 2 * NH * D), dtype=np.float32)
        for p in range(PAIRS):
            wqk_l[:, (2 * p) * P:(2 * p + 1) * P] = qcols[p]
            wqk_l[:, (2 * p + 1) * P:(2 * p + 2) * P] = kcols[p]
        bqk_full = np.empty(2 * NH * D, dtype=np.float32)
        for m in range(2 * PAIRS):
            bqk_full[m * P:(m + 1) * P] = bqk_l[m]
        wv_l = np.concatenate(
            [W_qkv[:, 2 * E + h * D:2 * E + (h + 1) * D] for h in heads], axis=1)
        bv_l = np.concatenate(
            [b_qkv[2 * E + h * D:2 * E + (h + 1) * D] for h in heads])
        wout_l = np.concatenate([W_out[h * D:(h + 1) * D, :] for h in heads],
                                axis=0)
        # pre-tile into the exact SBUF layouts for large-descriptor DMAs
        # (wqk m-chunk-major: [sbuf_partition, m, c, col])
        wqk_t = np.ascontiguousarray(
            wqk_l.reshape(NCC, P, 2 * PAIRS, P).transpose(1, 2, 0, 3)).astype(bf)
        wv_t = np.ascontiguousarray(
            wv_l.reshape(NCC, P, NH * D).transpose(1, 0, 2)).astype(bf)
        wout_t = np.ascontiguousarray(
            wout_l.reshape(PAIRS, P, E).transpose(1, 0, 2)).astype(bf)
        bqk_t = np.ascontiguousarray(
            bqk_full.reshape(NCC, P).T).astype(np.float32)
        btab_g = _build_bias_tabs(heads)
        slopes4_g = _slopes()[[heads[s] for s in range(4)]].astype(np.float32)
        per_group.append(dict(
            wqk=wqk_t, wv=wv_t, wout=wout_t,
            bqk=bqk_t, bv=bv_l.astype(bf),
            em=_build_em(heads), btab=btab_g, slopes4=slopes4_g))

    in_maps = []
    xt_cache = {}
    for c in range(N_CORES):
        b, g = c // 2, c % 2
        m = dict(per_group[g])
        if b not in xt_cache:
            xt = x[b].T  # [E, S]
            xt_cache[b] = np.ascontiguousarray(
                xt.reshape(NCC, P, NST, ST).transpose(2, 1, 0, 3)).astype(bf)
        m['xT'] = xt_cache[b]
        in_maps.append(m)
    return in_maps


def _run(inputs, trace=False, tmpdir=None, trace_cores=None):
    x = np.asarray(inputs['x'], dtype=np.float32)
    W_qkv = np.asarray(inputs['W_qkv'], dtype=np.float32)
    b_qkv = np.asarray(inputs['b_qkv'], dtype=np.float32)
    W_out = np.asarray(inputs['W_out'], dtype=np.float32)
    b_out = np.asarray(inputs['b_out'], dtype=np.float32)

    nc = _build_program()
    in_maps = _prep_core_inputs(x, W_qkv, b_qkv, W_out, b_out)
    res = bass_utils.run_bass_kernel_spmd(
        nc, in_maps, core_ids=list(range(N_CORES)), trace=trace, tmpdir=tmpdir,
        trace_cores=trace_cores)
    out = np.empty((B, S, E), dtype=np.float32)
    for b in range(B):
        out[b] = (res.results[2 * b]['out'].astype(np.float32)
                  + res.results[2 * b + 1]['out'].astype(np.float32) + b_out)
    return out, res


def kernel(**inputs) -> np.ndarray:
    out, _ = _run(inputs)
    return out
